# revision 1
# baseline (speedup 1.0000x reference)
"""MLA q/k/v projection kernel for Trainium2 (8 NeuronCores, token-data-parallel).

Self-contained: hardcodes the problem shapes from nn_MLA_81106162418389.
  hidden_state [2, 4096, 2048] f32 -> out [2, 16, 4096, 512] f32
Strategy: shard the 8192 tokens over 8 cores (1024 each); replicate weights.
Matmuls run in fp32r (tf32) mode; data pre-rounded to tf32 on host.
"""
import sys
sys.path.insert(0, "/opt/trn_rl_repo")

import numpy as np

import concourse.bass as bass
import concourse.tile as tile
from concourse import bacc, mybir
from concourse import bass2jax
from concourse.masks import make_identity

# ---- problem constants ----
HID, QK_NOPE, QK_ROPE, Q_LR, KV_LR, H, V_DIM = 2048, 128, 64, 768, 512, 16, 128
QK_HEAD = QK_NOPE + QK_ROPE           # 192
OUT_C = 2 * QK_HEAD + V_DIM           # 512
B, S = 2, 4096
THETA = 10000.0
EPS = 1e-5

N_CORES = 8
T = (B * S) // N_CORES                # 1024 tokens per core
P = 128
TCN = T // P                          # 8 token chunks
NT = 2                                # 512-wide token tiles for phase 1
KO = HID // P                         # 16 k-chunks for a-proj
ROQ = Q_LR // P                       # 6 r-chunks for q up-proj
ROKV = KV_LR // P                     # 4 r-chunks for kv up-proj
HH = H // 2                           # 8 heads per half-pass
NPF = 3                               # wq-A chunks prefetched during phase 1

F32 = mybir.dt.float32
R32 = mybir.dt.float32r
AF = mybir.ActivationFunctionType
OP = mybir.AluOpType


def _build(n_repeats=1, has_qb_bias=False, has_kvb_bias=False, variant="full"):
    nc = bacc.Bacc("TRN2", target_bir_lowering=False, debug=False,
                   num_devices=N_CORES)

    xT_d = nc.dram_tensor("xT", [HID, T], R32, kind="ExternalInput").ap()
    wqaT_d = nc.dram_tensor("wqaT", [HID, Q_LR], R32, kind="ExternalInput").ap()
    wkvaT_d = nc.dram_tensor("wkvaT", [HID, KV_LR + QK_ROPE], R32,
                             kind="ExternalInput").ap()
    wqbT_d = nc.dram_tensor("wqbT", [Q_LR, H * QK_HEAD], R32,
                            kind="ExternalInput").ap()
    wkvbT_d = nc.dram_tensor("wkvbT", [KV_LR, H * (QK_NOPE + V_DIM)], R32,
                             kind="ExternalInput").ap()
    bqa_d = nc.dram_tensor("bqa", [Q_LR], F32, kind="ExternalInput").ap()
    bkva_d = nc.dram_tensor("bkvap", [5 * P], F32, kind="ExternalInput").ap()
    cos_d = nc.dram_tensor("cosb", [T, QK_ROPE], F32, kind="ExternalInput").ap()
    sin_d = nc.dram_tensor("sinb", [T, QK_ROPE], F32, kind="ExternalInput").ap()
    if has_qb_bias:
        bqb_d = nc.dram_tensor("bqb", [H * QK_HEAD], F32, kind="ExternalInput").ap()
    if has_kvb_bias:
        bkvb_d = nc.dram_tensor("bkvb", [H * (QK_NOPE + V_DIM)], F32,
                                kind="ExternalInput").ap()
    ones_d = nc.dram_tensor("ones", [P, 1], R32, kind="ExternalInput").ap()
    out_d = nc.dram_tensor("out", [H, T, OUT_C], F32, kind="ExternalOutput").ap()

    for _ in range(n_repeats):
        _emit_once(nc, xT_d, wqaT_d, wkvaT_d, wqbT_d, wkvbT_d, bqa_d, bkva_d,
                   cos_d, sin_d,
                   bqb_d if has_qb_bias else None,
                   bkvb_d if has_kvb_bias else None,
                   out_d, ones_d, variant)
    nc.compile()
    return nc


def _emit_once(nc, xT_d, wqaT_d, wkvaT_d, wqbT_d, wkvbT_d, bqa_d, bkva_d,
               cos_d, sin_d, bqb_d, bkvb_d, out_d, ones_d, variant="full"):
    KVC = ROKV + 1          # 5 feature chunks for kv a-proj (last is 64-wide rope)
    with tile.TileContext(nc) as tc:
        with tc.tile_pool(name="persist", bufs=1) as pp, \
             tc.tile_pool(name="acts", bufs=1) as ap_pool:

            # ---- small persistent tiles ----
            cos_sb = pp.tile([P, TCN, QK_ROPE], F32)
            nc.sync.dma_start(cos_sb[:], cos_d.rearrange("(tc p) c -> p tc c", p=P))
            sin_sb = pp.tile([P, TCN, QK_ROPE], F32)
            nc.sync.dma_start(sin_sb[:], sin_d.rearrange("(tc p) c -> p tc c", p=P))
            bqa_sb = pp.tile([P, ROQ], F32)
            nc.sync.dma_start(bqa_sb[:], bqa_d.rearrange("(c p) -> p c", p=P))
            bkva_sb = pp.tile([P, 5], F32)
            nc.sync.dma_start(bkva_sb[:], bkva_d.rearrange("(c p) -> p c", p=P))
            ones_r = pp.tile([P, 1], R32)
            nc.sync.dma_start(ones_r[:], ones_d[:])
            eps_t = pp.tile([1, 1], F32)
            nc.gpsimd.memset(eps_t[:], EPS)
            ident = pp.tile([P, P], F32)
            make_identity(nc, ident[:])

            # activations (live across both phases)
            q_cT = ap_pool.tile([P, ROQ, T], R32)       # q low-rank, [feat, tok]
            kv_cT = ap_pool.tile([P, ROKV, T], R32)     # kv low-rank, [feat, tok]
            krot = ap_pool.tile([P, TCN, QK_ROPE], F32)  # roped k, [tok, chan]

            pf = None
            # ================= phase 1: a-projections + LN =================
            with tc.tile_pool(name="xw", bufs=1) as xw, \
                 tc.tile_pool(name="stat_sb", bufs=1) as st, \
                 tc.tile_pool(name="sq", bufs=2) as sqp, \
                 tc.tile_pool(name="ps1", bufs=3, space="PSUM") as ps1, \
                 tc.tile_pool(name="pst", bufs=3, space="PSUM") as pst, \
                 tc.tile_pool(name="ptr", bufs=1, space="PSUM") as ptr:

                krope = st.tile([64, T], F32)     # raw k rope slice [chan, tok]
                istd_bc = st.tile([P, T], F32, tag="istdbc", bufs=1)
                nmi_bc = st.tile([P, T], F32, tag="nmibc", bufs=1)

                x_t = {}
                wa_t = []
                for k in range(KO):
                    xt = xw.tile([P, NT, 512], R32, tag="x", bufs=KO,
                                 name=f"x_{k}")
                    nc.sync.dma_start(
                        xt[:], xT_d[k * P:(k + 1) * P, :].rearrange(
                            "p (nt t) -> p nt t", nt=NT))
                    for nt in range(NT):
                        x_t[(k, nt)] = xt[:, nt]
                    wt = xw.tile([P, Q_LR], R32, tag="wa", bufs=KO)
                    nc.sync.dma_start(wt[:], wqaT_d[k * P:(k + 1) * P, :])
                    wa_t.append(wt)
                if variant != "p1":
                    pf = []
                    for ro in range(NPF):
                        t = pp.tile([P, HH * QK_HEAD], R32, name=f"wq{ro}_pf",
                                    tag="pf", bufs=NPF)
                        nc.sync.dma_start(
                            t[:], wqbT_d[ro * P:(ro + 1) * P, 0:HH * QK_HEAD])
                        pf.append(t)

                # ---- mm1 q: q_cT[f, t] ----
                def _ln_stats(which, src_t, nfc, dim, norm_engines):
                    for nt in range(NT):
                        ts = slice(nt * 512, (nt + 1) * 512)
                        ps_s = pst.tile([1, 512], F32, name=f"pss_{which}{nt}",
                                        tag="pst", bufs=3)
                        for fc in range(nfc):
                            nc.tensor.matmul(ps_s[:], ones_r[:], src_t[:, fc, ts],
                                             start=(fc == 0), stop=(fc == nfc - 1))
                        ps_q = pst.tile([1, 512], F32, name=f"psq_{which}{nt}",
                                        tag="pst", bufs=3)
                        for fc in range(nfc):
                            sq = sqp.tile([P, 512], R32, tag="sq",
                                          name=f"sq_{which}{nt}{fc}")
                            nc.scalar.activation(sq[:], src_t[:, fc, ts], AF.Square)
                            nc.tensor.matmul(ps_q[:], ones_r[:], sq[:],
                                             start=(fc == 0), stop=(fc == nfc - 1))
                        mu = st.tile([1, 512], F32, tag="mu", bufs=1,
                                     name=f"mu_{which}{nt}")
                        istd = st.tile([1, 512], F32, tag="istd", bufs=1,
                                       name=f"istd_{which}{nt}")
                        nmi = st.tile([1, 512], F32, tag="nmi", bufs=1,
                                      name=f"nmi_{which}{nt}")
                        nc.scalar.mul(mu[:], ps_s[:], 1.0 / dim)
                        # istd <- mean(x^2) - mu^2
                        nc.vector.tensor_tensor(istd[:], mu[:], mu[:], OP.mult)
                        nc.vector.scalar_tensor_tensor(
                            istd[:], ps_q[:], 1.0 / dim, istd[:],
                            OP.mult, OP.subtract)
                        nc.scalar.activation(istd[:], istd[:], AF.Sqrt,
                                             bias=eps_t[:, 0:1])
                        nc.vector.reciprocal(istd[:], istd[:])
                        nc.vector.scalar_tensor_tensor(
                            nmi[:], mu[:], -1.0, istd[:], OP.mult, OP.mult)
                        nc.gpsimd.partition_broadcast(istd_bc[:, ts], istd[:])
                        nc.gpsimd.partition_broadcast(nmi_bc[:, ts], nmi[:])
                    # normalize in place (writes tf32)
                    for fc in range(nfc):
                        eng = norm_engines[fc % len(norm_engines)]
                        eng.tensor_tensor(src_t[:, fc, :], src_t[:, fc, :],
                                          istd_bc[:], OP.mult)
                        eng.tensor_tensor(src_t[:, fc, :], src_t[:, fc, :],
                                          nmi_bc[:], OP.add)

                for nt in range(NT):
                    ts = slice(nt * 512, (nt + 1) * 512)
                    for g in range(2):          # groups of 3 psums
                        pss = [ps1.tile([P, 512], F32, name=f"ps1_{nt}_{g}_{fi}", tag="ps1", bufs=4) for fi in range(3)]
                        for k in range(KO):
                            for fi in range(3):
                                fc = g * 3 + fi
                                nc.tensor.matmul(
                                    pss[fi][:], wa_t[k][:, fc * P:(fc + 1) * P],
                                    x_t[(k, nt)][:],
                                    start=(k == 0), stop=(k == KO - 1))
                        for fi in range(3):
                            fc = g * 3 + fi
                            nc.scalar.activation(q_cT[:, fc, ts], pss[fi][:],
                                                 AF.Identity,
                                                 bias=bqa_sb[:, fc:fc + 1])

                # ---- q LN (PE stat matmuls run before mm1-kv; DVE
                # normalize overlaps mm1-kv on PE) ----
                _ln_stats("q", q_cT, ROQ, Q_LR, (nc.vector,))

                # ---- mm1 kv (reuse "wa" slots) ----
                wkv_t = []
                for k in range(KO):
                    wt = xw.tile([P, KV_LR + QK_ROPE], R32, tag="wa", bufs=KO,
                                 name=f"wkv_{k}")
                    nc.sync.dma_start(wt[:], wkvaT_d[k * P:(k + 1) * P, :])
                    wkv_t.append(wt)
                for nt in range(NT):
                    ts = slice(nt * 512, (nt + 1) * 512)
                    for g, nfc in ((0, 3), (1, 2)):
                        pss = [ps1.tile([P, 512], F32, name=f"ps1kv_{nt}_{g}_{fi}", tag="ps1", bufs=4) for fi in range(nfc)]
                        for k in range(KO):
                            for fi in range(nfc):
                                fc = g * 3 + fi
                                w = 64 if fc == 4 else P
                                nc.tensor.matmul(
                                    pss[fi][:w],
                                    wkv_t[k][:, fc * P:fc * P + w],
                                    x_t[(k, nt)][:],
                                    start=(k == 0), stop=(k == KO - 1))
                        for fi in range(nfc):
                            fc = g * 3 + fi
                            if fc == 4:
                                nc.scalar.activation(krope[:, ts], pss[fi][:64],
                                                     AF.Identity,
                                                     bias=bkva_sb[:64, 4:5])
                            else:
                                nc.scalar.activation(kv_cT[:, fc, ts], pss[fi][:],
                                                     AF.Identity,
                                                     bias=bkva_sb[:, fc:fc + 1])

                # ---- kv LN (normalize split DVE/gpsimd) ----
                _ln_stats("kv", kv_cT, ROKV, KV_LR, (nc.vector, nc.gpsimd))

                # ---- k rope: transpose [64, T] -> [T, 64], then rotate ----
                krope_t = st.tile([P, TCN, QK_ROPE], F32)
                for tci in range(TCN):
                    ps_t = ptr.tile([P, 64], F32)
                    nc.tensor.transpose(ps_t[:], krope[:, tci * P:(tci + 1) * P],
                                        ident[:64, :64])
                    nc.scalar.copy(krope_t[:, tci, :], ps_t[:])
                tmp = st.tile([P, TCN, 32], F32)
                tmp2 = st.tile([P, TCN, 32], F32)
                nc.vector.tensor_tensor(tmp[:], krope_t[:, :, 32:64],
                                        sin_sb[:, :, 0:32], OP.mult)
                nc.vector.tensor_tensor(tmp2[:], krope_t[:, :, 0:32],
                                        sin_sb[:, :, 32:64], OP.mult)
                nc.vector.tensor_tensor(krot[:], krope_t[:], cos_sb[:], OP.mult)
                nc.vector.tensor_tensor(krot[:, :, 0:32], krot[:, :, 0:32],
                                        tmp[:], OP.subtract)
                nc.vector.tensor_tensor(krot[:, :, 32:64], krot[:, :, 32:64],
                                        tmp2[:], OP.add)

            # ================= phase 2: up-projections + assemble =============
            if variant == "p1":
                return
            with tc.tile_pool(name="wb", bufs=1) as wb, \
                 tc.tile_pool(name="outp", bufs=1) as outp, \
                 tc.tile_pool(name="rsc", bufs=3) as rsc, \
                 tc.tile_pool(name="bias2", bufs=1) as bias2, \
                 tc.tile_pool(name="ps2", bufs=1, space="PSUM") as ps2:

                QW = HH * QK_HEAD            # 1536 q cols per half
                KW = HH * (QK_NOPE + V_DIM)  # 2048 kv cols per half
                outT = out_d.rearrange("h t c -> t h c")

                bqb_bc = bkvb_bc = None
                if bqb_d is not None:
                    b1 = bias2.tile([1, H * QK_HEAD], F32)
                    nc.sync.dma_start(b1[:], bqb_d[None, :])
                    bqb_bc = bias2.tile([P, H * QK_HEAD], F32)
                    nc.gpsimd.partition_broadcast(bqb_bc[:], b1[:])
                if bkvb_d is not None:
                    b2 = bias2.tile([1, H * (QK_NOPE + V_DIM)], F32)
                    nc.sync.dma_start(b2[:], bkvb_d[None, :])
                    bkvb_bc = bias2.tile([P, H * (QK_NOPE + V_DIM)], F32)
                    nc.gpsimd.partition_broadcast(bkvb_bc[:], b2[:])

                # ---- up-proj passes, interleaved q/kv per half so each
                # pass's weights fully prefetch during the previous pass ----
                for kind, half in (("q", 0), ("kv", 0), ("q", 1), ("kv", 1)):
                    if kind == "q":
                        h0 = half * HH
                        c0q = h0 * QK_HEAD
                        wq_t = []
                        for ro in range(ROQ):
                            if half == 0 and ro < NPF:
                                wq_t.append(pf[ro])
                                continue
                            wt = wb.tile([P, QW], R32, tag="wq", bufs=ROQ + 3,
                                         name=f"wq_{half}_{ro}")
                            nc.sync.dma_start(
                                wt[:], wqbT_d[ro * P:(ro + 1) * P, c0q:c0q + QW])
                            wq_t.append(wt)

                        for tci in range(TCN):
                            tsl = slice(tci * P, (tci + 1) * P)
                            ob = outp.tile([P, HH, QK_HEAD], F32, tag="obq", bufs=2,
                                           name=f"obq_{half}_{tci}")
                            obv = ob.rearrange("p (i j) c -> p i j c", j=2)
                            for s in range(2):
                                isl = slice(2 * s, 2 * s + 2)
                                psq = ps2.tile([P, 2, 512], F32,
                                               name=f"psq_{half}_{tci}_{s}",
                                               tag="ps2", bufs=4)
                                for i in range(2):
                                    gi = 2 * s + i
                                    for ro in range(ROQ):
                                        nc.tensor.matmul(
                                            psq[:, i, 0:2 * QK_HEAD], q_cT[:, ro, tsl],
                                            wq_t[ro][:, gi * 2 * QK_HEAD:
                                                      (gi + 1) * 2 * QK_HEAD],
                                            start=(ro == 0), stop=(ro == ROQ - 1))
                                if bqb_bc is not None:
                                    nc.vector.tensor_tensor(
                                        psq[:, :, 0:2 * QK_HEAD],
                                        psq[:, :, 0:2 * QK_HEAD],
                                        bqb_bc[:, c0q + s * 768:c0q + s * 768 + 768
                                               ].rearrange("p (i c) -> p i c", c=384),
                                        OP.add)
                                for j in range(2):
                                    nc.scalar.copy(obv[:, isl, j, 0:QK_NOPE],
                                                   psq[:, :, j * QK_HEAD:
                                                       j * QK_HEAD + QK_NOPE])
                                cosb = cos_sb[:, tci:tci + 1, None, :].to_broadcast(
                                    [P, 2, 2, QK_ROPE])
                                sinb = sin_sb[:, tci:tci + 1, None, :].to_broadcast(
                                    [P, 2, 2, QK_ROPE])
                                xr = psq[:, :, 0:2 * QK_HEAD].rearrange(
                                    "p i (j c) -> p i j c", c=QK_HEAD)[
                                    :, :, :, QK_NOPE:QK_HEAD]
                                orp = obv[:, isl, :, QK_NOPE:QK_HEAD]
                                t1 = rsc.tile([P, 2, 2, 32], F32, tag="t1",
                                              name=f"t1_{half}_{tci}_{s}")
                                t2 = rsc.tile([P, 2, 2, 32], F32, tag="t2",
                                              name=f"t2_{half}_{tci}_{s}")
                                nc.vector.tensor_tensor(t1[:], xr[:, :, :, 32:64],
                                                        sinb[:, :, :, 0:32], OP.mult)
                                nc.vector.tensor_tensor(t2[:], xr[:, :, :, 0:32],
                                                        sinb[:, :, :, 32:64], OP.mult)
                                nc.vector.tensor_tensor(orp[:], xr[:], cosb[:],
                                                        OP.mult)
                                nc.vector.tensor_tensor(orp[:, :, :, 0:32],
                                                        orp[:, :, :, 0:32], t1[:],
                                                        OP.subtract)
                                nc.vector.tensor_tensor(orp[:, :, :, 32:64],
                                                        orp[:, :, :, 32:64], t2[:],
                                                        OP.add)
                            if variant != "nostores":
                                for s2 in range(2):
                                    nc.sync.dma_start(
                                        outT[tsl, h0 + 4 * s2:h0 + 4 * s2 + 4,
                                             0:QK_HEAD], ob[:, 4 * s2:4 * s2 + 4])
                    else:
                        h0 = half * HH
                        c0kv = h0 * (QK_NOPE + V_DIM)
                        wkv_t = []
                        for ro in range(ROKV):
                            wt = wb.tile([P, KW], R32, tag="wkv", bufs=ROKV + 2,
                                         name=f"wkv_{half}_{ro}")
                            nc.sync.dma_start(
                                wt[:], wkvbT_d[ro * P:(ro + 1) * P, c0kv:c0kv + KW])
                            wkv_t.append(wt)

                        for tci in range(TCN):
                            tsl = slice(tci * P, (tci + 1) * P)
                            ob = outp.tile([P, HH, OUT_C - QK_HEAD], F32, tag="obkv",
                                           bufs=2, name=f"obkv_{half}_{tci}")
                            obv = ob.rearrange("p (i j) c -> p i j c", j=2)
                            for s in range(2):
                                isl = slice(2 * s, 2 * s + 2)
                                pskv = ps2.tile([P, 2, 512], F32,
                                                name=f"pskv_{half}_{tci}_{s}",
                                                tag="ps2", bufs=4)
                                for i in range(2):
                                    gi = 2 * s + i
                                    for ro in range(ROKV):
                                        nc.tensor.matmul(
                                            pskv[:, i, :], kv_cT[:, ro, tsl],
                                            wkv_t[ro][:, gi * 512:(gi + 1) * 512],
                                            start=(ro == 0), stop=(ro == ROKV - 1))
                                if bkvb_bc is not None:
                                    nc.vector.tensor_tensor(
                                        pskv[:], pskv[:],
                                        bkvb_bc[:, c0kv + s * 1024:
                                                c0kv + s * 1024 + 1024
                                                ].rearrange("p (i c) -> p i c", c=512),
                                        OP.add)
                                for j in range(2):
                                    # k_nope -> local cols 0:128 (global 192:320)
                                    nc.scalar.copy(
                                        obv[:, isl, j, 0:QK_NOPE],
                                        pskv[:, :, j * 256:j * 256 + QK_NOPE])
                                    # v -> local cols 192:320 (global 384:512)
                                    nc.vector.tensor_copy(
                                        obv[:, isl, j, QK_NOPE + QK_ROPE:],
                                        pskv[:, :, j * 256 + QK_NOPE:(j + 1) * 256])
                            # k_rot -> local cols 128:192 (global 320:384)
                            nc.scalar.copy(
                                ob[:, :, QK_NOPE:QK_NOPE + QK_ROPE],
                                krot[:, tci:tci + 1, :].to_broadcast(
                                    [P, HH, QK_ROPE]))
                            if variant != "nostores":
                                for s2 in range(2):
                                    nc.sync.dma_start(
                                        outT[tsl, h0 + 4 * s2:h0 + 4 * s2 + 4,
                                             QK_HEAD:OUT_C], ob[:, 4 * s2:4 * s2 + 4])

# ------------------------- host side -------------------------

def _round_tf32(x):
    u = np.ascontiguousarray(x).view(np.uint32).astype(np.uint64)
    u = (u + 0xFFF + ((u >> 13) & 1)) >> 13 << 13
    return (u & 0xFFFFFFFF).astype(np.uint32).view(np.float32)


def _rope_tables(s0):
    pos = np.arange(s0, s0 + T, dtype=np.float64)
    inv = 1.0 / THETA ** (np.arange(0, QK_ROPE, 2, dtype=np.float64) / QK_ROPE)
    fr = pos[:, None] * inv[None, :]
    cos = np.concatenate([np.cos(fr), np.cos(fr)], axis=1).astype(np.float32)
    sin = np.concatenate([np.sin(fr), np.sin(fr)], axis=1).astype(np.float32)
    return cos, sin


_prog_cache = {}


def kernel(hidden_state, w_qa, b_qa, g_qa_ln, b_qa_ln, w_qb, b_qb,
           w_kva, b_kva, g_kva_ln, b_kva_ln, w_kvb, b_kvb):
    hidden_state = np.asarray(hidden_state, dtype=np.float32)
    w_qa = np.asarray(w_qa, dtype=np.float32)
    w_qb = np.asarray(w_qb, dtype=np.float32)
    w_kva = np.asarray(w_kva, dtype=np.float32)
    w_kvb = np.asarray(w_kvb, dtype=np.float32)
    b_qa = np.asarray(b_qa, dtype=np.float32)
    b_kva = np.asarray(b_kva, dtype=np.float32)
    g_qa_ln = np.asarray(g_qa_ln, dtype=np.float32)
    b_qa_ln = np.asarray(b_qa_ln, dtype=np.float32)
    g_kva_ln = np.asarray(g_kva_ln, dtype=np.float32)
    b_kva_ln = np.asarray(b_kva_ln, dtype=np.float32)
    b_qb = np.asarray(b_qb, dtype=np.float32)
    b_kvb = np.asarray(b_kvb, dtype=np.float32)

    wqaT = _round_tf32(w_qa.T)
    wkvaT = _round_tf32(w_kva.T)
    wqbT = _round_tf32((w_qb * g_qa_ln[None, :]).T)
    wkvbT = _round_tf32((w_kvb * g_kva_ln[None, :]).T)
    bqb_eff = (b_qb + w_qb @ b_qa_ln).astype(np.float32)
    bkvb_eff = (b_kvb + w_kvb @ b_kva_ln).astype(np.float32)
    bkva_pad = np.zeros(5 * P, np.float32)
    bkva_pad[:KV_LR + QK_ROPE] = b_kva

    has_qb = bool(np.any(bqb_eff))
    has_kvb = bool(np.any(bkvb_eff))
    key = (has_qb, has_kvb)
    if key not in _prog_cache:
        _prog_cache[key] = _build(1, has_qb, has_kvb)
    nc = _prog_cache[key]

    flat = hidden_state.reshape(B * S, HID)
    in_maps = []
    for c in range(N_CORES):
        tok0 = c * T
        s0 = tok0 % S
        cos, sin = _rope_tables(s0)
        m = {
            "xT": _round_tf32(flat[tok0:tok0 + T].T),
            "wqaT": wqaT, "wkvaT": wkvaT, "wqbT": wqbT, "wkvbT": wkvbT,
            "bqa": b_qa, "bkvap": bkva_pad, "cosb": cos, "sinb": sin,
            "ones": np.ones((P, 1), np.float32),
        }
        if has_qb:
            m["bqb"] = bqb_eff
        if has_kvb:
            m["bkvb"] = bkvb_eff
        in_maps.append(m)

    res = bass2jax.run_bass_via_pjrt(nc, in_maps, n_cores=N_CORES)

    out = np.empty((B, H, S, OUT_C), np.float32)
    for c in range(N_CORES):
        tok0 = c * T
        b = tok0 // S
        s0 = tok0 % S
        out[b, :, s0:s0 + T, :] = res[c]["out"]
    return out



# revision 63
# speedup vs baseline: 6.9274x; 6.9274x over previous
"""MLA q/k/v projection kernel for Trainium2 (8 NeuronCores, token-data-parallel).

Self-contained: hardcodes the problem shapes from nn_MLA_81106162418389.
  hidden_state [2, 4096, 2048] f32 -> out [2, 16, 4096, 512] f32
Strategy: shard the 8192 tokens over 8 cores (1024 each); replicate weights.
All matmul operands in bf16; single persistent pool structure (no phase
barriers); PSUM = 3x[P,2,512] + 2x[1,512] ring (8 banks exactly).
"""
import sys
sys.path.insert(0, "/opt/trn_rl_repo")

import numpy as np
import ml_dtypes

import concourse.bass as bass
import concourse.tile as tile
from concourse import bacc, mybir
from concourse import bass2jax
from concourse.masks import make_identity


# ---- problem constants ----
HID, QK_NOPE, QK_ROPE, Q_LR, KV_LR, H, V_DIM = 2048, 128, 64, 768, 512, 16, 128
QK_HEAD = QK_NOPE + QK_ROPE           # 192
OUT_C = 2 * QK_HEAD + V_DIM           # 512
B, S = 2, 4096
THETA = 10000.0
EPS = 1e-5

N_CORES = 8
T = (B * S) // N_CORES                # 1024 tokens per core
P = 128
TCN = T // P                          # 8 token chunks
NT = 2                                # 512-wide token tiles for phase 1
KO = HID // P                         # 16 k-chunks for a-proj
ROQ = Q_LR // P                       # 6 r-chunks for q up-proj
ROKV = KV_LR // P                     # 4 r-chunks for kv up-proj
HH = H // 2                           # 8 heads per half-pass
QW = HH * QK_HEAD                     # 1536 q cols per half
KW = HH * (QK_NOPE + V_DIM)           # 2048 kv cols per half

F32 = mybir.dt.float32
BF16 = mybir.dt.bfloat16
AF = mybir.ActivationFunctionType
OP = mybir.AluOpType


def _build(n_repeats=1, has_qb_bias=False, has_kvb_bias=False):
    nc = bacc.Bacc("TRN2", target_bir_lowering=False, debug=False,
                   num_devices=N_CORES)

    xT_d = nc.dram_tensor("xT", [HID, T], BF16, kind="ExternalInput").ap()
    wqaT_d = nc.dram_tensor("wqaT", [HID, Q_LR], BF16, kind="ExternalInput").ap()
    wkvaT_d = nc.dram_tensor("wkvaT", [HID, KV_LR + QK_ROPE], BF16,
                             kind="ExternalInput").ap()
    wqbT_d = nc.dram_tensor("wqbT", [Q_LR, H * QK_HEAD], BF16,
                            kind="ExternalInput").ap()
    wkvbT_d = nc.dram_tensor("wkvbT", [KV_LR, H * (QK_NOPE + V_DIM)], BF16,
                             kind="ExternalInput").ap()
    bqa_d = nc.dram_tensor("bqa", [P, ROQ], F32, kind="ExternalInput").ap()
    bkva_d = nc.dram_tensor("bkvap", [P, 5], F32, kind="ExternalInput").ap()
    cos_d = nc.dram_tensor("cosb", [P, TCN, QK_ROPE], F32,
                           kind="ExternalInput").ap()
    sin_d = nc.dram_tensor("sinb", [P, TCN, QK_ROPE], F32,
                           kind="ExternalInput").ap()
    bqb_d = bkvb_d = None
    if has_qb_bias:
        bqb_d = nc.dram_tensor("bqb", [H * QK_HEAD], F32, kind="ExternalInput").ap()
    if has_kvb_bias:
        bkvb_d = nc.dram_tensor("bkvb", [H * (QK_NOPE + V_DIM)], F32,
                                kind="ExternalInput").ap()
    out_d = nc.dram_tensor("out", [H, T, OUT_C], F32, kind="ExternalOutput").ap()
    ist_d = nc.dram_tensor("ist_scratch", [2, T], F32, kind="Internal").ap()

    for _ in range(n_repeats):
        _emit_once(nc, xT_d, wqaT_d, wkvaT_d, wqbT_d, wkvbT_d, bqa_d, bkva_d,
                   cos_d, sin_d, bqb_d, bkvb_d, out_d, ist_d)
    nc.compile()
    return nc


def _emit_once(nc, xT_d, wqaT_d, wkvaT_d, wqbT_d, wkvbT_d, bqa_d, bkva_d,
               cos_d, sin_d, bqb_d, bkvb_d, out_d, ist_d):
    with tile.TileContext(nc) as tc:
        with tc.tile_pool(name="pp", bufs=1) as pp, \
             tc.tile_pool(name="ws", bufs=1) as ws, \
             tc.tile_pool(name="psp", bufs=1, space="PSUM") as psp:

            # ---- persistent smalls (x0/wa0 jump the DMA queue below) ----
            bqa_sb = pp.tile([P, ROQ], F32)
            bkva_sb = pp.tile([P, 5], F32)
            cos_sb = pp.tile([P, TCN, QK_ROPE], F32)
            sin_sb = pp.tile([P, TCN, QK_ROPE], F32)
            bqb_bc = bkvb_bc = None
            if bqb_d is not None:
                b1 = pp.tile([1, H * QK_HEAD], F32)
                nc.sync.dma_start(b1[:], bqb_d[None, :])
                bqb_bc = pp.tile([P, H * QK_HEAD], F32)
                nc.gpsimd.partition_broadcast(bqb_bc[:], b1[:])
            if bkvb_d is not None:
                b2 = pp.tile([1, H * (QK_NOPE + V_DIM)], F32)
                nc.sync.dma_start(b2[:], bkvb_d[None, :])
                bkvb_bc = pp.tile([P, H * (QK_NOPE + V_DIM)], F32)
                nc.gpsimd.partition_broadcast(bkvb_bc[:], b2[:])

            ones_b = pp.tile([P, 1], BF16)
            nc.gpsimd.memset(ones_b[:], 1.0)
            eps_t = pp.tile([1, 1], F32)
            nc.gpsimd.memset(eps_t[:], EPS)
            ident = pp.tile([P, P], F32)
            make_identity(nc, ident[:])

            # ---- persistent activations ----
            q_cT = pp.tile([P, ROQ, T], BF16)
            kv_cT = pp.tile([P, ROKV, T], BF16)
            krope = pp.tile([64, T], F32)
            krope_t = pp.tile([P, TCN, QK_ROPE], F32)
            krot = pp.tile([P, TCN, QK_ROPE], F32)
            nbc_q = pp.tile([P, T], BF16)
            nbc_kv = pp.tile([P, T], BF16)
            istq_t = pp.tile([P, TCN], F32)
            istkv_t = pp.tile([P, TCN], F32)
            istq_row = pp.tile([1, T], F32)
            istkv_row = pp.tile([1, T], F32)

            # ---- bulk loads, in consumption order on one queue ----
            x_t, wa_t = [], []
            for k in range(KO):
                xt = ws.tile([P, T], BF16, tag="x", bufs=KO, name=f"x_{k}")
                nc.sync.dma_start(xt[:], xT_d[k * P:(k + 1) * P, :])
                x_t.append(xt)
                wt = ws.tile([P, Q_LR], BF16, tag="wa", bufs=KO,
                             name=f"wa_{k}")
                nc.sync.dma_start(wt[:], wqaT_d[k * P:(k + 1) * P, :])
                wa_t.append(wt)
                if k == 0:
                    nc.sync.dma_start(bqa_sb[:], bqa_d[:])
                    nc.sync.dma_start(bkva_sb[:], bkva_d[:])
            wkva_big = ws.tile([P, KO, KV_LR + QK_ROPE], BF16, tag="wkva",
                               bufs=1, name="wkva")
            nc.sync.dma_start(
                wkva_big[:], wkvaT_d.rearrange("(k p) c -> p k c", p=P))
            wkva_t = [wkva_big[:, k] for k in range(KO)]
            nc.sync.dma_start(cos_sb[:], cos_d[:])
            nc.sync.dma_start(sin_sb[:], sin_d[:])
            wq_big = ws.tile([P, ROQ, H * QK_HEAD], BF16, tag="wq", bufs=1,
                             name="wq")
            nc.sync.dma_start(
                wq_big[:], wqbT_d.rearrange("(r p) c -> p r c", p=P))
            wq_t = {(half, ro): wq_big[:, ro,
                                       half * QW:(half + 1) * QW]
                    for half in range(2) for ro in range(ROQ)}
            # kv up-proj weights ride the freed x slots (loads unblock as
            # mm1 consumes x); they are last in the queue so nothing stalls
            # behind them.
            wkv_t = {}
            for half in range(2):
                c0kv = half * KW
                for ro in range(ROKV):
                    for piece in range(2):
                        t = ws.tile([P, T], BF16, tag="x", bufs=KO,
                                    name=f"wkv_{half}_{ro}_{piece}")
                        nc.sync.dma_start(
                            t[:], wkvbT_d[ro * P:(ro + 1) * P,
                                          c0kv + piece * 1024:
                                          c0kv + (piece + 1) * 1024])
                        wkv_t[(half, ro, piece)] = t

            # ================= phase 1: a-projections + LN =================
            def _mm1(w_tiles, dst, dst_bias, groups, rope_fc=None):
                for fcs in groups:
                    tiles = {}
                    for fc in fcs:
                        tiles[fc] = psp.tile([P, NT, 512], F32, tag="big",
                                             bufs=3, name=f"mm1_{fc}")
                    for k in range(KO):
                        for fc in fcs:
                            rows = 64 if fc == rope_fc else P
                            for nt in range(NT):
                                nc.tensor.matmul(
                                    tiles[fc][0:rows, nt, :],
                                    w_tiles[k][:, fc * P:fc * P + rows],
                                    x_t[k][:, nt * 512:(nt + 1) * 512],
                                    start=(k == 0), stop=(k == KO - 1))
                    for fc in fcs:
                        if fc == rope_fc:
                            nc.scalar.activation(
                                krope[:].rearrange("q (nt t) -> q nt t", nt=NT),
                                tiles[fc][0:64], AF.Identity,
                                bias=bkva_sb[0:64, 4:5])
                        else:
                            nc.scalar.activation(
                                dst[:, fc, :].rearrange("p (nt t) -> p nt t",
                                                        nt=NT),
                                tiles[fc][:], AF.Identity,
                                bias=dst_bias[:, fc:fc + 1])

            _mm1(wa_t, q_cT, bqa_sb, ((0, 1, 2), (3, 4, 5)))

            def _ln_sq(src, nfc, which):
                # precompute squares on DVE so the PE stat matmuls never wait
                sqs = {}
                for nt in range(NT):
                    nts = slice(nt * 512, (nt + 1) * 512)
                    for fc in range(nfc):
                        sq = ws.tile([P, 512], BF16, tag="sq", bufs=8,
                                     name=f"sq_{which}_{nt}_{fc}")
                        nc.vector.tensor_tensor(sq[:], src[:, fc, nts],
                                                src[:, fc, nts], OP.mult)
                        sqs[(nt, fc)] = sq
                return sqs

            def _ln(src, nfc, dim, nbc, ist_t, istd_row, sqs):
                # mean-subtract src in place; 1/std goes to ist_t in
                # token-major layout, applied later as a per-partition scale
                # on the phase-2 copies.
                for nt in range(NT):
                    nts = slice(nt * 512, (nt + 1) * 512)
                    ps_s = psp.tile([1, 512], F32, tag="st", bufs=2,
                                    name=f"pss_{nt}")
                    for fc in range(nfc):
                        nc.tensor.matmul(ps_s[:], ones_b[:], src[:, fc, nts],
                                         start=(fc == 0), stop=(fc == nfc - 1))
                    ps_q = psp.tile([1, 512], F32, tag="st", bufs=2,
                                    name=f"psq_{nt}")
                    for fc in range(nfc):
                        nc.tensor.matmul(ps_q[:], ones_b[:], sqs[(nt, fc)][:],
                                         start=(fc == 0), stop=(fc == nfc - 1))
                    mu = ws.tile([1, 512], F32, tag="mu", bufs=1,
                                 name=f"mu_{nt}")
                    istd = istd_row[:, nts]
                    nc.scalar.mul(mu[:], ps_s[:], 1.0 / dim)
                    nc.vector.tensor_tensor(istd, mu[:], mu[:], OP.mult)
                    nc.vector.scalar_tensor_tensor(
                        istd, ps_q[:], 1.0 / dim, istd,
                        OP.mult, OP.subtract)
                    nc.scalar.activation(istd, istd, AF.Sqrt,
                                         bias=eps_t[:, 0:1])
                    nc.vector.reciprocal_approx_fast(istd, istd)
                    nh = ws.tile([1, 512], BF16, tag="nh", bufs=2,
                                 name=f"nh_{nt}")
                    nc.vector.tensor_scalar_mul(nh[:], mu[:], -1.0)
                    nc.gpsimd.partition_broadcast(nbc[:, nts], nh[:])
                # istd [1, T] -> token-major [P, TCN] via a DRAM round-trip
                # (an SBUF source AP cannot remap free-axis strides onto
                # partitions; a DRAM source can). Same queue => ordered.
                # Scalar queue: the sync queue is clogged by slot-blocked
                # wkv load issues until mm1kv drains.
                row = ist_d[0 if ist_t is istq_t else 1]
                nc.scalar.dma_start(row[None, :], istd_row[:])
                nc.scalar.dma_start(
                    ist_t[:], row.rearrange("(tc p) -> p tc", p=P))
                for fc in range(nfc):
                    nc.vector.tensor_tensor(src[:, fc, :], src[:, fc, :],
                                            nbc[:], OP.add)

            _ln(q_cT, ROQ, Q_LR, nbc_q, istq_t, istq_row,
                _ln_sq(q_cT, ROQ, "q"))

            _mm1(wkva_t, kv_cT, bkva_sb, ((0, 1, 2), (3, 4)), rope_fc=4)
            kv_sqs = _ln_sq(kv_cT, ROKV, "kv")

            def _krope_block():
                ptile = psp.tile([P, NT, 512], F32, tag="big", bufs=3,
                                 name="ptr")
                for tci in range(TCN):
                    nc.tensor.transpose(
                        ptile[:, tci // 4, (tci % 4) * 64:(tci % 4) * 64 + 64],
                        krope[:, tci * P:(tci + 1) * P], ident[:64, :64])
                nc.scalar.copy(
                    krope_t[:].rearrange("p (a b) c -> p a b c", a=2),
                    ptile[:, :, 0:256].rearrange("p a (b c) -> p a b c", c=64))
                tmp = ws.tile([P, TCN, 32], F32, tag="krtmp", bufs=2,
                              name="tmp")
                tmp2 = ws.tile([P, TCN, 32], F32, tag="krtmp", bufs=2,
                               name="tmp2")
                nc.vector.tensor_tensor(tmp[:], krope_t[:, :, 32:64],
                                        sin_sb[:, :, 0:32], OP.mult)
                nc.vector.tensor_tensor(tmp2[:], krope_t[:, :, 0:32],
                                        sin_sb[:, :, 32:64], OP.mult)
                nc.vector.tensor_tensor(krot[:], krope_t[:], cos_sb[:],
                                        OP.mult)
                nc.vector.tensor_tensor(krot[:, :, 0:32], krot[:, :, 0:32],
                                        tmp[:], OP.subtract)
                nc.vector.tensor_tensor(krot[:, :, 32:64], krot[:, :, 32:64],
                                        tmp2[:], OP.add)

            # ================= phase 2: up-projections + assemble ==========
            outT = out_d.rearrange("h t c -> t h c")

            def _q_pass(half, tcis=range(TCN)):
                h0 = half * HH
                c0q = half * QW
                pending = []
                for tci in tcis:
                    tsl = slice(tci * P, (tci + 1) * P)
                    for s in range(2):
                        ob = ws.tile([P, 4, QK_HEAD], F32, tag="obq", bufs=4,
                                     name=f"obq_{half}_{tci}_{s}")
                        obv = ob.rearrange("p (i j) c -> p i j c", j=2)
                        psq = psp.tile([P, NT, 512], F32, tag="big", bufs=3,
                                       name=f"psq_{half}_{tci}_{s}")
                        for i in range(2):
                            gi = 2 * s + i
                            for ro in range(ROQ):
                                nc.tensor.matmul(
                                    psq[:, i, 0:2 * QK_HEAD],
                                    q_cT[:, ro, tsl],
                                    wq_t[(half, ro)][:, gi * 2 * QK_HEAD:
                                                     (gi + 1) * 2 * QK_HEAD],
                                    start=(ro == 0), stop=(ro == ROQ - 1))
                        src = psq[:, :, 0:2 * QK_HEAD].rearrange(
                            "p i (j c) -> p i j c", c=QK_HEAD)
                        nc.scalar.activation(obv[:], src[:], AF.Identity,
                                             scale=istq_t[:, tci:tci + 1])
                        if bqb_bc is not None:
                            nc.vector.tensor_tensor(
                                ob[:], ob[:],
                                bqb_bc[:, c0q + s * 768:c0q + s * 768 + 768
                                       ].rearrange("p (i c) -> p i c", c=192),
                                OP.add)
                        # rope in place on SBUF, 4 heads at a time
                        orp = ob[:, :, QK_NOPE:QK_HEAD]
                        cosb = cos_sb[:, tci:tci + 1, :].to_broadcast(
                            [P, 4, QK_ROPE])
                        sinb = sin_sb[:, tci:tci + 1, :].to_broadcast(
                            [P, 4, QK_ROPE])
                        t1 = ws.tile([P, 4, 32], F32, tag="t1", bufs=2,
                                     name=f"t1_{half}_{tci}_{s}")
                        t2 = ws.tile([P, 4, 32], F32, tag="t2", bufs=2,
                                     name=f"t2_{half}_{tci}_{s}")
                        nc.vector.tensor_tensor(t1[:], orp[:, :, 32:64],
                                                sinb[:, :, 0:32], OP.mult)
                        nc.vector.tensor_tensor(t2[:], orp[:, :, 0:32],
                                                sinb[:, :, 32:64], OP.mult)
                        nc.vector.tensor_tensor(orp[:], orp[:], cosb[:],
                                                OP.mult)
                        nc.vector.tensor_tensor(orp[:, :, 0:32],
                                                orp[:, :, 0:32], t1[:],
                                                OP.subtract)
                        nc.vector.tensor_tensor(orp[:, :, 32:64],
                                                orp[:, :, 32:64], t2[:],
                                                OP.add)
                        # defer the store issue one group so scalar never
                        # blocks waiting on the vector rope of this group
                        pending.append(
                            (outT[tsl, h0 + 4 * s:h0 + 4 * s + 4, 0:QK_HEAD],
                             ob))
                        if len(pending) > 1:
                            dst, srct = pending.pop(0)
                            nc.scalar.dma_start(dst, srct[:])
                for dst, srct in pending:
                    nc.scalar.dma_start(dst, srct[:])

            def _kv_pass(half, store_eng=None):
                h0 = half * HH
                c0kv = half * KW
                store_eng = store_eng or nc.scalar
                pending = []
                for tci in range(TCN):
                    tsl = slice(tci * P, (tci + 1) * P)
                    for s in range(2):
                        ob = ws.tile([P, 4, OUT_C - QK_HEAD], F32, tag="obkv",
                                     bufs=4, name=f"obkv_{half}_{tci}_{s}")
                        obv = ob.rearrange("p (i j) c -> p i j c", j=2)
                        pskv = psp.tile([P, NT, 512], F32, tag="big", bufs=3,
                                        name=f"pskv_{half}_{tci}_{s}")
                        for i in range(2):
                            gi = 2 * s + i
                            for ro in range(ROKV):
                                nc.tensor.matmul(
                                    pskv[:, i, :], kv_cT[:, ro, tsl],
                                    wkv_t[(half, ro, gi // 2)][
                                        :, (gi % 2) * 512:(gi % 2) * 512 + 512],
                                    start=(ro == 0), stop=(ro == ROKV - 1))
                        src = pskv[:].rearrange("p i (j c) -> p i j c", c=256)
                        # k_nope -> local cols 0:128 (global 192:320)
                        nc.scalar.activation(obv[:, :, :, 0:QK_NOPE],
                                             src[:, :, :, 0:QK_NOPE],
                                             AF.Identity,
                                             scale=istkv_t[:, tci:tci + 1])
                        # v -> local cols 192:320 (global 384:512)
                        nc.vector.tensor_scalar_mul(
                            obv[:, :, :, QK_NOPE + QK_ROPE:],
                            src[:, :, :, QK_NOPE:256],
                            istkv_t[:, tci:tci + 1])
                        if bkvb_bc is not None:
                            bsl = bkvb_bc[:, c0kv + s * 1024:
                                          c0kv + s * 1024 + 1024
                                          ].rearrange(
                                "p (i j two c) -> p i j two c",
                                i=2, two=2, c=128)
                            nc.vector.tensor_tensor(
                                obv[:, :, :, 0:QK_NOPE],
                                obv[:, :, :, 0:QK_NOPE],
                                bsl[:, :, :, 0, :], OP.add)
                            nc.vector.tensor_tensor(
                                obv[:, :, :, QK_NOPE + QK_ROPE:],
                                obv[:, :, :, QK_NOPE + QK_ROPE:],
                                bsl[:, :, :, 1, :], OP.add)
                        # k_rot -> local cols 128:192 (global 320:384)
                        nc.vector.tensor_copy(
                            ob[:, :, QK_NOPE:QK_NOPE + QK_ROPE],
                            krot[:, tci:tci + 1, :].to_broadcast(
                                [P, 4, QK_ROPE]))
                        pending.append(
                            (outT[tsl, h0 + 4 * s:h0 + 4 * s + 4,
                                  QK_HEAD:OUT_C], ob))
                        if len(pending) > 1:
                            dst, srct = pending.pop(0)
                            store_eng.dma_start(dst, srct[:])
                for dst, srct in pending:
                    store_eng.dma_start(dst, srct[:])

            # PE order: a few q tcis first so the kv stat matmuls (which wait
            # on vector squares) and kv LN overlap with q up-proj matmuls.
            _q_pass(0, range(0, 3))
            _krope_block()
            _ln(kv_cT, ROKV, KV_LR, nbc_kv, istkv_t, istkv_row, kv_sqs)
            _q_pass(0, range(3, TCN))
            _kv_pass(0)
            _kv_pass(1)
            _q_pass(1)

# ------------------------- host side -------------------------

def _bf16(x):
    return np.ascontiguousarray(x).astype(ml_dtypes.bfloat16)


def _rope_tables(s0):
    pos = np.arange(s0, s0 + T, dtype=np.float64)
    inv = 1.0 / THETA ** (np.arange(0, QK_ROPE, 2, dtype=np.float64) / QK_ROPE)
    fr = pos[:, None] * inv[None, :]
    cos = np.concatenate([np.cos(fr), np.cos(fr)], axis=1).astype(np.float32)
    sin = np.concatenate([np.sin(fr), np.sin(fr)], axis=1).astype(np.float32)
    # [T, 64] -> [P, TCN, 64] with token t = tc*128 + p
    return (cos.reshape(TCN, P, QK_ROPE).transpose(1, 0, 2).copy(),
            sin.reshape(TCN, P, QK_ROPE).transpose(1, 0, 2).copy())


def build_in_maps(inputs):
    f32 = np.float32
    w_qa = np.asarray(inputs["w_qa"], f32)
    w_qb = np.asarray(inputs["w_qb"], f32)
    w_kva = np.asarray(inputs["w_kva"], f32)
    w_kvb = np.asarray(inputs["w_kvb"], f32)
    g_qa_ln = np.asarray(inputs["g_qa_ln"], f32)
    b_qa_ln = np.asarray(inputs["b_qa_ln"], f32)
    g_kva_ln = np.asarray(inputs["g_kva_ln"], f32)
    b_kva_ln = np.asarray(inputs["b_kva_ln"], f32)
    b_qa = np.asarray(inputs["b_qa"], f32)
    b_kva = np.asarray(inputs["b_kva"], f32)
    b_qb = np.asarray(inputs["b_qb"], f32)
    b_kvb = np.asarray(inputs["b_kvb"], f32)

    wqaT = _bf16(w_qa.T)
    wkvaT = _bf16(w_kva.T)
    wqbT = _bf16((w_qb * g_qa_ln[None, :]).T)
    wkvbT = _bf16((w_kvb * g_kva_ln[None, :]).T)
    bqb_eff = (b_qb + w_qb @ b_qa_ln).astype(f32)
    bkvb_eff = (b_kvb + w_kvb @ b_kva_ln).astype(f32)
    bqa_arr = b_qa.reshape(ROQ, P).T.copy()          # [P, 6]
    bkva_pad = np.zeros(5 * P, f32)
    bkva_pad[:KV_LR + QK_ROPE] = b_kva
    bkva_arr = bkva_pad.reshape(5, P).T.copy()       # [P, 5]

    has_qb = bool(np.any(bqb_eff))
    has_kvb = bool(np.any(bkvb_eff))

    flat = np.asarray(inputs["hidden_state"], f32).reshape(B * S, HID)
    in_maps = []
    for c in range(N_CORES):
        tok0 = c * T
        cos, sin = _rope_tables(tok0 % S)
        m = {
            "xT": _bf16(flat[tok0:tok0 + T].T),
            "wqaT": wqaT, "wkvaT": wkvaT, "wqbT": wqbT, "wkvbT": wkvbT,
            "bqa": bqa_arr, "bkvap": bkva_arr, "cosb": cos, "sinb": sin,
        }
        if has_qb:
            m["bqb"] = bqb_eff
        if has_kvb:
            m["bkvb"] = bkvb_eff
        in_maps.append(m)
    return in_maps, has_qb, has_kvb


_prog_cache = {}


def kernel(hidden_state, w_qa, b_qa, g_qa_ln, b_qa_ln, w_qb, b_qb,
           w_kva, b_kva, g_kva_ln, b_kva_ln, w_kvb, b_kvb):
    inputs = dict(hidden_state=hidden_state, w_qa=w_qa, b_qa=b_qa,
                  g_qa_ln=g_qa_ln, b_qa_ln=b_qa_ln, w_qb=w_qb, b_qb=b_qb,
                  w_kva=w_kva, b_kva=b_kva, g_kva_ln=g_kva_ln,
                  b_kva_ln=b_kva_ln, w_kvb=w_kvb, b_kvb=b_kvb)
    in_maps, has_qb, has_kvb = build_in_maps(inputs)
    key = (has_qb, has_kvb)
    if key not in _prog_cache:
        _prog_cache[key] = _build(1, has_qb, has_kvb)
    nc = _prog_cache[key]

    res = bass2jax.run_bass_via_pjrt(nc, in_maps, n_cores=N_CORES)

    out = np.empty((B, H, S, OUT_C), np.float32)
    for c in range(N_CORES):
        tok0 = c * T
        b = tok0 // S
        s0 = tok0 % S
        out[b, :, s0:s0 + T, :] = res[c]["out"]
    return out
